# revision 43
# baseline (speedup 1.0000x reference)
"""CGCNN forward pass on 8 Trainium2 NeuronCores (Bass/Tile).

Key algebraic identity exploited: the reference uses row = edge_index[0] for
BOTH the gather (h[row]) and the scatter (segment_sum(..., row)), so

    segment_sum(h[row] * ef, row) == h * segment_sum(ef, row)

i.e. the per-edge gather of node features disappears entirely, and the edge
MLP outputs for all 4 conv layers (which depend only on edge_attr) can be
computed in a single pass with stacked weights [64, 4*128].

Sharding: nodes (and the edges that scatter into them) are partitioned into 8
contiguous ranges of 5000 nodes -> one range per core.  Every core's work is
fully independent (no collectives); the only cross-core reduction is the final
mean pool, done on host over 8 [128]-vectors, followed by the (tiny) dense
head in numpy.

Device pipeline per core (edges grouped into 128-edge tiles per 128-node
window, two tiles = one "pair", two pairs = one "block"):
  - EF:    per tile ef_ps[128e, 512] = eaT.T @ Wstack  (two tile_position-
           packed K=64 matmuls per pair when be == 0; K=65 bias-row fallback)
  - evict: ef_pair = relu(ef_ps) cast fp8; Scalar takes one tile, Vector the
           other (swapped each pair) so both engines run every pair and the
           PSUM banks recycle at half the single-engine latency
  - scatter (lagged): blocks of two DoubleRow fp8 matmuls back to back
           (S_psum[128n, 512] += onehot.T @ ef_pair); blocking amortizes the
           256-column DR LDWEIGHTS which cannot hide under EF streams
  - per window: evict S (Scalar), 4x PE-transpose into one [128,512] bf16
    PSUM tile, single strided Vector copy to S^T storage
  - node stage in [h, n] layout: h = W_emb^T x^T (K padded to 128), then 4x
    (hs = h*S_l on Vector bf16 2x; u = Wn_l^T hs (N=512); relu+bias on
    Scalar; affine on GpSimd; h += u on Vector), BN folded into per-partition
    affine A,B.  Mean-pool partial on Vector -> [128,1].
"""

import os
import sys

for _p in ("/opt/trn_rl_repo",):
    if _p not in sys.path and os.path.isdir(_p):
        sys.path.insert(0, _p)

import numpy as np
import ml_dtypes

import concourse.bass as bass  # noqa: F401
import concourse.mybir as mybir
import concourse.tile as tile
from concourse import bacc
from concourse.bass_utils import run_bass_kernel_spmd

N, E = 40000, 640000
NODE_F, EDGE_F, H = 92, 64, 128
L_CONV, L_DENSE = 4, 2
EPS = 1e-3

NCORES = 8
NPC = N // NCORES              # 5000 nodes per core
WIN = 128                      # node window (psum partition dim)
NWIN = (NPC + WIN - 1) // WIN  # 40
NPAD = NWIN * WIN              # 5120
HS = L_CONV * H                # 512 stacked hidden
BF16 = ml_dtypes.bfloat16

_cache = {}


def _host_prep(x, edge_index, edge_attr, W_emb, b_emb, We, be, packed):
    """Bucket+sort edges by destination node, pad to 128-edge tiles per
    128-node window (uniform tile counts across cores, NT even), build
    one-hot tiles and transposed inputs, all bf16."""
    row = np.asarray(edge_index)[0].astype(np.int64)
    ea = np.asarray(edge_attr, np.float32)

    # Degree-balanced node->(core,window) assignment: nodes are permutation-
    # invariant (only the global sum matters), so bin-pack them into the 320
    # buckets (cap 128 nodes) equalizing per-bucket edge counts.  This makes
    # every window need the same tile count (Tw uniform+even -> no window-
    # straddling pairs) and removes ~4% padding.
    import heapq
    deg = np.bincount(row, minlength=N).astype(np.int64)
    NBUCK = NCORES * NWIN
    nodes_by_deg = np.argsort(-deg, kind="stable")
    heap = [(0, b) for b in range(NBUCK)]
    heapq.heapify(heap)
    bucket_of = np.zeros(N, np.int32)
    bucket_cnt = np.zeros(NBUCK, np.int32)
    col_of = np.zeros(N, np.int32)
    spill = []
    for nd in nodes_by_deg:
        s, b = heapq.heappop(heap)
        bucket_of[nd] = b
        col_of[nd] = bucket_cnt[b]
        bucket_cnt[b] += 1
        if bucket_cnt[b] < WIN:
            heapq.heappush(heap, (s + int(deg[nd]), b))
        else:
            spill.append((s + int(deg[nd]), b))
    node_core = (bucket_of // NWIN).astype(np.int64)
    node_win = (bucket_of % NWIN).astype(np.int64)

    core = node_core[row]
    win = node_win[row]
    col = col_of[row].astype(np.int32)
    key = (core * NWIN + win).astype(np.int64)

    order = np.argsort(key, kind="stable")
    counts = np.bincount(key, minlength=NCORES * NWIN).reshape(NCORES, NWIN)
    Tw = np.maximum(np.ceil(counts.max(axis=0) / 128).astype(np.int64), 1)
    if Tw.sum() % 2:
        Tw[-1] += 1  # keep NT even so tiles pair cleanly
    NT = int(Tw.sum())
    EP = NT * 128
    tile_start = np.zeros(NWIN, np.int64)
    tile_start[1:] = np.cumsum(Tw)[:-1]
    slot_start = tile_start * 128

    seg_end = np.cumsum(counts.reshape(-1))
    seg_start = seg_end - counts.reshape(-1)

    ea64 = np.zeros((NCORES, 64, EP), np.float32)
    colslot = np.full((NCORES, EP), -1, np.int32)
    for c in range(NCORES):
        for w in range(NWIN):
            k = c * NWIN + w
            idx = order[seg_start[k]:seg_end[k]]
            n = len(idx)
            if n == 0:
                continue
            s0 = slot_start[w]
            ea64[c, :, s0:s0 + n] = ea[idx].T
            colslot[c, s0:s0 + n] = col[idx]

    if packed:
        # pair layout: rows 0:64 = even tile features, 64:128 = odd tile
        a = ea64.reshape(NCORES, 64, NT, 128)
        eat = np.concatenate(
            [a[:, :, 0::2, :].reshape(NCORES, 64, EP // 2),
             a[:, :, 1::2, :].reshape(NCORES, 64, EP // 2)], axis=1)
    else:
        eat = np.concatenate(
            [ea64, np.ones((NCORES, 1, EP), np.float32)], axis=1)  # bias row

    # one-hot, layout [core, e_in_tile(partition), tile, node_in_window]
    oh = np.zeros((NCORES, 128, NT, 128), np.float32)
    cc, ss = np.nonzero(colslot >= 0)
    t_idx = ss // 128
    e_in = ss - t_idx * 128
    oh[cc, e_in, t_idx, colslot[cc, ss]] = 1.0
    oh = oh.reshape(NCORES, 128, NT * 128)

    # node features transposed + ones row, padded to NPAD; K padded to 128
    xt = np.zeros((NCORES, 128, NPAD), np.float32)
    xf = np.asarray(x, np.float32)
    loc = node_win * WIN + col_of
    xt[node_core, :NODE_F, loc] = xf
    xt[:, 92, :] = 1.0
    nreal = np.bincount(node_core, minlength=NCORES)

    wemb128 = np.zeros((128, 128), np.float32)
    wemb128[:NODE_F] = np.asarray(W_emb, np.float32)
    wemb128[92] = np.asarray(b_emb, np.float32)

    Wef = np.asarray(We, np.float32)
    bef = np.asarray(be, np.float32)
    if packed:
        wstack = np.zeros((128, HS), np.float32)
        for l in range(L_CONV):
            wstack[:64, l * H:(l + 1) * H] = Wef[l]
            wstack[64:, l * H:(l + 1) * H] = Wef[l]
    else:
        wstack = np.zeros((65, HS), np.float32)
        for l in range(L_CONV):
            wstack[:64, l * H:(l + 1) * H] = Wef[l]
            wstack[64, l * H:(l + 1) * H] = bef[l]

    import concourse.mybir as _mb
    F8 = _mb.dt.np(_mb.dt.float8e4)
    return {
        "NT": NT, "Tw": Tw, "EP": EP, "nreal": nreal,
        "eat": np.ascontiguousarray(eat).astype(BF16),
        "oh": oh.astype(F8),
        "xt": xt.astype(BF16),
        "wemb": wemb128.astype(BF16),
        "wstack": wstack.astype(BF16),
    }


def _build_program(NT, Tw, packed):
    from concourse.masks import make_identity

    EP = NT * 128
    f32 = mybir.dt.float32
    bf = mybir.dt.bfloat16
    f8 = mybir.dt.float8e4
    Relu = mybir.ActivationFunctionType.Relu
    MULT = mybir.AluOpType.mult
    ADD = mybir.AluOpType.add

    nc = bacc.Bacc(None, target_bir_lowering=False)
    d_eat = nc.dram_tensor(
        "eat", [128 if packed else 65, EP // 2 if packed else EP], bf,
        kind="ExternalInput")
    d_oh = nc.dram_tensor("oh", [128, EP], f8, kind="ExternalInput")
    d_xt = nc.dram_tensor("xt", [128, NPAD], bf, kind="ExternalInput")
    d_wemb = nc.dram_tensor("wemb", [128, 128], bf, kind="ExternalInput")
    d_wstack = nc.dram_tensor(
        "wstack", [128 if packed else 65, HS], bf, kind="ExternalInput")
    d_wn = nc.dram_tensor("wn", [128, HS], bf, kind="ExternalInput")
    d_bnb = nc.dram_tensor("bnb", [128, L_CONV], f32, kind="ExternalInput")
    d_ab = nc.dram_tensor("ab", [128, L_CONV], f32, kind="ExternalInput")
    d_abn = nc.dram_tensor("abn", [128, L_CONV], f32, kind="ExternalInput")
    d_bb = nc.dram_tensor("bb", [128, L_CONV], bf, kind="ExternalInput")
    d_out = nc.dram_tensor("hsum", [128, 1], f32, kind="ExternalOutput")

    NPAIR = NT // 2
    # node-chunk schedule: 512-wide chunks, the last 512 split into 128s to
    # shorten the serial dependency chain in the kernel tail
    CHUNKS = []
    _ci = 0
    _n = 0
    while _n < NPAD:
        w_ = 128 if _n >= NPAD - 512 else 512
        CHUNKS.append((_ci, _n, _n + w_))
        _ci += 1
        _n += w_
    N_CHUNKS = len(CHUNKS)
    GP = 8                      # pairs per DMA group (16 tiles)
    LAG = 10                    # scatter lags LAG pairs
    FLUSH = 4                   # scatter block size (pairs per flush)

    # window id per tile
    win_of = np.repeat(np.arange(NWIN), Tw)
    first_of = np.zeros(NT, bool)
    last_of = np.zeros(NT, bool)
    pos = 0
    for w in range(NWIN):
        first_of[pos] = True
        last_of[pos + int(Tw[w]) - 1] = True
        pos += int(Tw[w])

    with tile.TileContext(nc) as tc:
        with (
            tc.tile_pool(name="const", bufs=1) as constp,
            tc.tile_pool(name="ea", bufs=4) as eap,
            tc.tile_pool(name="ohp", bufs=4) as ohp,
            tc.tile_pool(name="ef", bufs=20) as efp,
            tc.tile_pool(name="swin", bufs=3) as swinp,
            tc.tile_pool(name="efps", bufs=4, space="PSUM") as efpsp,
            tc.tile_pool(name="sps", bufs=2, space="PSUM") as spsp,
            tc.tile_pool(name="big", bufs=1) as bigp,
        ):
            # first edge-data slice goes FIRST so the PE can start ~5us
            # earlier; everything else queues behind it
            gs0 = min(GP, NPAIR)
            ea_t0 = eap.tile(
                [128, GP * 128] if packed else [65, GP * 256], bf,
                tag="ea", name="ea_t0")
            h0 = min(1, gs0)
            if packed:
                nc.sync.dma_start(ea_t0[:, :h0 * 128], d_eat[:, :h0 * 128])
            else:
                nc.sync.dma_start(ea_t0[:, :h0 * 256], d_eat[:, :h0 * 256])
            wstack_sb = constp.tile([128 if packed else 65, HS], bf)
            nc.sync.dma_start(wstack_sb[:], d_wstack[:])
            oh_t0 = ohp.tile([128, GP * 256], f8, tag="oh", name="oh_t0")
            nc.sync.dma_start(oh_t0[:, :h0 * 256], d_oh[:, :h0 * 256])
            if packed and gs0 > h0:
                nc.sync.dma_start(
                    ea_t0[:, h0 * 128:gs0 * 128],
                    d_eat[:, h0 * 128:gs0 * 128])
            elif gs0 > h0:
                nc.sync.dma_start(
                    ea_t0[:, h0 * 256:gs0 * 256],
                    d_eat[:, h0 * 256:gs0 * 256])
            if gs0 > h0:
                nc.sync.dma_start(
                    oh_t0[:, h0 * 256:gs0 * 256],
                    d_oh[:, h0 * 256:gs0 * 256])
            wemb_sb = constp.tile([128, 128], bf)
            nc.sync.dma_start(wemb_sb[:], d_wemb[:])
            wn_sb = constp.tile([128, HS], bf)
            nc.sync.dma_start(wn_sb[:], d_wn[:])
            bnb_sb = constp.tile([128, L_CONV], f32)
            nc.sync.dma_start(bnb_sb[:], d_bnb[:])
            ab_sb = constp.tile([128, L_CONV], f32)
            nc.sync.dma_start(ab_sb[:], d_ab[:])
            abn_sb = constp.tile([128, L_CONV], f32)
            nc.sync.dma_start(abn_sb[:], d_abn[:])
            bb_sb = constp.tile([128, L_CONV], bf)
            nc.sync.dma_start(bb_sb[:], d_bb[:])
            ident_sb = constp.tile([128, 128], bf)
            make_identity(nc, ident_sb[:])

            xt_sb = bigp.tile([128, NPAD], bf)
            NK = NPAD // 512
            st_sb = bigp.tile([128, L_CONV * NPAD], bf)   # S^T per layer
            h_sb = bigp.tile([128, NPAD], bf)
            hs_sb = bigp.tile([128, NPAD], bf)
            t_sb = bigp.tile([128, NPAD], bf)
            u_sb = bigp.tile([128, NPAD], bf)
            hsum_sb = bigp.tile([128, 1], f32)
            hpart_sb = bigp.tile([128, N_CHUNKS], f32)

            node_chains = []
            tr_queue = []
            pop_rr = [0]

            def make_emb(n0, n1):
                def s_emb():
                    cw = n1 - n0
                    h_ps = spsp.tile(
                        [128, 512], f32, tag="nodeps", bufs=1, name="h_ps")
                    nc.tensor.matmul(
                        h_ps[:, :cw], wemb_sb[:], xt_sb[:, n0:n1],
                        start=True, stop=True)
                    nc.scalar.copy(h_sb[:, n0:n1], h_ps[:, :cw])
                return s_emb

            def pop_steps(n):
                for _ in range(n):
                    # transposes drain first: chunk chains depend on their
                    # windows' S^T being written (program order guarantees it)
                    if tr_queue:
                        tr_queue.pop(0)()
                        continue
                    if not node_chains:
                        return
                    idx = pop_rr[0] % min(2, len(node_chains))
                    chain = node_chains[idx]
                    chain.pop(0)()
                    if not chain:
                        node_chains.pop(idx)
                    pop_rr[0] += 1

            def queue_node_chunk(k, n0, n1):
                """Embed + all 4 conv layers for node cols [n0, n1), as a
                list of small steps drip-fed so dependent chains never
                block the in-order engine FIFOs.  hs on Vector (bf16 2x),
                relu(+folded BN scale) on Scalar, h update fused on
                Vector, matmuls on TensorE."""
                sl = slice(n0, n1)
                cw = n1 - n0
                chain = [make_emb(n0, n1)]

                def s_mul(l):
                    def f():
                        nc.vector.tensor_tensor(
                            hs_sb[:, sl], h_sb[:, sl],
                            st_sb[:, l * NPAD + n0:l * NPAD + n1],
                            op=MULT)
                    return f

                def s_mm(l):
                    def f():
                        u_ps = spsp.tile(
                            [128, 512], f32, tag="nodeps", bufs=1,
                            name="u_ps")
                        nc.tensor.matmul(
                            u_ps[:, :cw], wn_sb[:, l * 128:(l + 1) * 128],
                            hs_sb[:, sl], start=True, stop=True)
                        # A>0 so A*relu(u+bn) == relu(A*u + A*bn): BN scale
                        # folds into the relu eviction for free
                        nc.scalar.activation(
                            t_sb[:, sl], u_ps[:, :cw], Relu,
                            bias=abn_sb[:, l:l + 1],
                            scale=ab_sb[:, l:l + 1])
                    return f

                def s_stt(l):
                    def f():
                        # h += t + B in one fused op; the final layer also
                        # emits the chunk's mean-pool partial via accum_out
                        nc.vector.scalar_tensor_tensor(
                            h_sb[:, sl], t_sb[:, sl], bb_sb[:, l:l + 1],
                            h_sb[:, sl], op0=ADD, op1=ADD,
                            accum_out=(hpart_sb[:, k:k + 1]
                                       if l == L_CONV - 1 else None))
                    return f

                for l in range(L_CONV):
                    chain.append(s_mul(l))
                    chain.append(s_mm(l))
                    chain.append(s_stt(l))

                node_chains.append(chain)

            # ---------------- edge stage (software-pipelined) ------------
            state = {"s_ps": None}
            ef_tiles = [None] * NPAIR
            oh_groups = [None] * NPAIR

            def scatter_pair_dr(q, ef_pair, oh_g, goff):
                """Both tiles of pair q in one fp8 DoubleRow matmul
                (contracts 256 edges at 2 MAC/cell/cycle)."""
                t0 = 2 * q
                if first_of[t0]:
                    state["s_ps"] = spsp.tile(
                        [128, HS], f32, tag="sps", name="s_ps")
                s_ps = state["s_ps"]
                j = t0 - goff
                oh3 = oh_g[:, j * 128:(j + 2) * 128].rearrange(
                    "p (two n) -> p two n", two=2)
                ef3 = ef_pair[:].rearrange("p (two n) -> p two n", two=2)
                nc.tensor.matmul(
                    s_ps[:], oh3, ef3,
                    start=bool(first_of[t0]), stop=bool(last_of[t0 + 1]),
                    perf_mode=mybir.MatmulPerfMode.DoubleRow)
                finish_window(t0 + 1)

            def emit_scatter(t, ef_pair, oh_g, goff):
                if first_of[t]:
                    state["s_ps"] = spsp.tile(
                        [128, HS], f32, tag="sps", name="s_ps")
                s_ps = state["s_ps"]
                j = t - goff
                nc.tensor.matmul(
                    s_ps[:], oh_g[:, j * 128:(j + 1) * 128],
                    ef_pair[:, (t % 2) * HS:(t % 2 + 1) * HS],
                    start=bool(first_of[t]), stop=bool(last_of[t]))
                finish_window(t)

            def finish_window(t):
                w = int(win_of[t])
                s_ps = state["s_ps"]
                if last_of[t]:
                    s_sb = swinp.tile([128, HS], bf, tag="swin")
                    nc.scalar.copy(s_sb[:], s_ps[:])

                    def s_tr(s_sb, w, l, tr_box):
                        def f():
                            if l == 0:
                                tr_box[0] = spsp.tile(
                                    [128, 512], bf, tag="trps", bufs=1,
                                    name="tr_ps")
                            tr_ps = tr_box[0]
                            nc.tensor.transpose(
                                tr_ps[:, l * 128:(l + 1) * 128],
                                s_sb[:, l * 128:(l + 1) * 128],
                                ident_sb[:])
                            if l == L_CONV - 1:
                                # one strided copy moves all 4 transposed
                                # layers into S^T storage
                                dst = st_sb[:].rearrange(
                                    "p (l m) -> p l m", l=L_CONV)[
                                    :, :, w * WIN:(w + 1) * WIN]
                                nc.vector.tensor_copy(dst, tr_ps[:].rearrange(
                                    "p (l n) -> p l n", l=L_CONV))
                        return f
                    tr_box = [None]
                    tr_queue.extend(
                        [s_tr(s_sb, w, l, tr_box) for l in range(L_CONV)])
                    done = (w + 1) * WIN
                    while CHUNKS and CHUNKS[0][2] <= done:
                        queue_node_chunk(*CHUNKS.pop(0))

            parity = 0
            g0p = 0
            ns_box = [0]
            pend_scatter = []
            group_tiles = {}

            def load_group(p0):
                if p0 >= NPAIR or p0 in group_tiles:
                    return
                gs = min(GP, NPAIR - p0)
                ea_t = eap.tile(
                    [128, GP * 128] if packed else [65, GP * 256], bf,
                    tag="ea", name="ea_t")
                if packed:
                    nc.sync.dma_start(
                        ea_t[:, :gs * 128],
                        d_eat[:, p0 * 128:(p0 + gs) * 128])
                else:
                    nc.sync.dma_start(
                        ea_t[:, :gs * 256],
                        d_eat[:, p0 * 256:(p0 + gs) * 256])
                oh_t = ohp.tile([128, GP * 256], f8, tag="oh", name="oh_t")
                nc.sync.dma_start(
                    oh_t[:, :gs * 256], d_oh[:, p0 * 256:(p0 + gs) * 256])
                group_tiles[p0] = (ea_t, oh_t)

            group_tiles[0] = (ea_t0, oh_t0)
            load_group(GP)
            for p in range(NPAIR):
                if p < NK:
                    nc.sync.dma_start(
                        xt_sb[:, p * 512:(p + 1) * 512],
                        d_xt[:, p * 512:(p + 1) * 512])
                if p % GP == 0:
                    g0p = p
                    ea_g, oh_g = group_tiles[p]
                    load_group(p + 2 * GP)   # keep two groups in flight

                jp = p - g0p
                ps_a = efpsp.tile([128, HS], f32, tag="efps", name="ef_psa")
                ps_b = efpsp.tile([128, HS], f32, tag="efps", name="ef_psb")
                if packed:
                    pe = ea_g[:, jp * 128:(jp + 1) * 128]
                    nc.tensor.matmul(
                        ps_a[:], pe[0:64, :], wstack_sb[0:64, :],
                        start=True, stop=True, tile_position=(0, 0))
                    nc.tensor.matmul(
                        ps_b[:], pe[64:128, :],
                        wstack_sb[64:128, :],
                        start=True, stop=True, tile_position=(64, 0))
                else:
                    nc.tensor.matmul(
                        ps_a[:], ea_g[:, jp * 256:jp * 256 + 128],
                        wstack_sb[:], start=True, stop=True)
                    nc.tensor.matmul(
                        ps_b[:],
                        ea_g[:, jp * 256 + 128:(jp + 1) * 256],
                        wstack_sb[:], start=True, stop=True)

                ef_pair = efp.tile([128, 2 * HS], f8, tag="ef")
                # both engines evict every pair (one tile each, swapping
                # roles) so PSUM banks free at half the single-engine latency
                if parity:
                    nc.scalar.activation(ef_pair[:, 0:HS], ps_a[:], Relu)
                    nc.vector.tensor_scalar_max(
                        ef_pair[:, HS:2 * HS], ps_b[:], 0.0)
                else:
                    nc.vector.tensor_scalar_max(ef_pair[:, 0:HS], ps_a[:], 0.0)
                    nc.scalar.activation(ef_pair[:, HS:2 * HS], ps_b[:], Relu)
                parity ^= 1
                ef_tiles[p] = ef_pair
                oh_groups[p] = (oh_g, 2 * g0p)

                lag_eff = 3 if (p < 12 or p >= NPAIR - 6) else LAG
                while ns_box[0] <= p - lag_eff:
                    pend_scatter.append(ns_box[0])
                    ns_box[0] += 1
                flushed = False
                if len(pend_scatter) >= (
                        2 if (p < 12 or p >= NPAIR - 6) else FLUSH) or (
                        p == NPAIR - 1 and pend_scatter):
                    for q in pend_scatter:
                        og, goff = oh_groups[q]
                        if win_of[2 * q] == win_of[2 * q + 1]:
                            scatter_pair_dr(q, ef_tiles[q], og, goff)
                        else:
                            emit_scatter(2 * q, ef_tiles[q], og, goff)
                            emit_scatter(2 * q + 1, ef_tiles[q], og, goff)
                        ef_tiles[q] = None
                    pend_scatter = []
                    flushed = True
                # burst node steps right after scatter blocks so their PE
                # matmuls cluster with the DR run instead of breaking up EF
                if flushed or NPAIR - p < 60:
                    pop_steps(FLUSH + 1 if flushed else 2)
            for q in range(ns_box[0], NPAIR):
                og, goff = oh_groups[q]
                if win_of[2 * q] == win_of[2 * q + 1]:
                    scatter_pair_dr(q, ef_tiles[q], og, goff)
                else:
                    emit_scatter(2 * q, ef_tiles[q], og, goff)
                    emit_scatter(2 * q + 1, ef_tiles[q], og, goff)
            pop_steps(10 ** 9)

            nc.vector.tensor_reduce(
                hsum_sb[:], hpart_sb[:], axis=mybir.AxisListType.X,
                op=ADD)
            nc.sync.dma_start(d_out[:], hsum_sb[:])

    nc.finalize()
    return nc


LAST_EXEC_NS = None
LAST_RESULT = None


def kernel(x, edge_index, edge_attr, W_emb, b_emb, We, be, Wn, bn,
           g_c, beta_c, mu_c, var_c, Wd, bd, g_d, beta_d, mu_d, var_d, Wf, bf):
    global LAST_EXEC_NS

    packed = bool(np.all(np.asarray(be) == 0.0))
    prep = _host_prep(x, edge_index, edge_attr, W_emb, b_emb, We, be, packed)
    NT, Tw = prep["NT"], prep["Tw"]

    Wnf = np.asarray(Wn, np.float32)
    wn_stack = np.zeros((128, HS), np.float32)
    for l in range(L_CONV):
        wn_stack[:, l * H:(l + 1) * H] = Wnf[l]
    A = (np.asarray(g_c, np.float32)
         / np.sqrt(np.asarray(var_c, np.float32) + EPS))        # [L, H]
    B = np.asarray(beta_c, np.float32) - np.asarray(mu_c, np.float32) * A

    key = (NT, tuple(int(v) for v in Tw), packed)
    if key not in _cache:
        _cache[key] = _build_program(NT, Tw, packed)
    nc = _cache[key]

    common = {
        "wemb": prep["wemb"],
        "wstack": prep["wstack"],
        "wn": wn_stack.astype(BF16),
        "bnb": np.ascontiguousarray(np.asarray(bn, np.float32).T).reshape(128, L_CONV),
        "ab": np.ascontiguousarray(A.T).reshape(128, L_CONV),
        "abn": np.ascontiguousarray((A * np.asarray(bn, np.float32)).T).reshape(128, L_CONV),
        "bb": np.ascontiguousarray(B.T).reshape(128, L_CONV).astype(BF16),
    }
    in_maps = []
    for c in range(NCORES):
        m = dict(common)
        m["eat"] = prep["eat"][c]
        m["oh"] = prep["oh"][c]
        m["xt"] = prep["xt"][c]
        in_maps.append(m)

    trace = bool(os.environ.get("KERNEL_TRACE"))
    if trace:
        try:
            from trn_agent_boot.trn_boot import _ntff_profile_via_ctypes
            from antenv.axon_hooks import set_axon_ntff_profile_hook
            set_axon_ntff_profile_hook(
                _ntff_profile_via_ctypes("/opt/axon/libaxon_pjrt.so"))
        except Exception:
            trace = False

    res = run_bass_kernel_spmd(
        nc, in_maps, core_ids=list(range(NCORES)), trace=trace)
    LAST_EXEC_NS = res.exec_time_ns
    global LAST_RESULT
    LAST_RESULT = res

    total = np.zeros(128, np.float64)
    for c in range(NCORES):
        total += res.results[c]["hsum"].reshape(128).astype(np.float64)
    # padded (fake) node columns contribute a closed-form constant: S=0 there
    # so h_fake = b_emb + sum_l (A_l*relu(bn_l) + B_l)
    h_fake = np.asarray(b_emb, np.float64).copy()
    for l in range(L_CONV):
        h_fake = h_fake + A[l].astype(np.float64) * np.maximum(
            np.asarray(bn, np.float64)[l], 0.0) + B[l].astype(np.float64)
    n_fake = NCORES * NPAD - N
    total -= n_fake * h_fake
    v = (total / N).astype(np.float32)

    # dense head on host (0.000001% of total FLOPs)
    g_df = np.asarray(g_d, np.float32)
    var_df = np.asarray(var_d, np.float32)
    beta_df = np.asarray(beta_d, np.float32)
    mu_df = np.asarray(mu_d, np.float32)
    Wdf = np.asarray(Wd, np.float32)
    bdf = np.asarray(bd, np.float32)
    for d in range(L_DENSE):
        v = np.maximum(v @ Wdf[d] + bdf[d], 0.0)
        Ad = g_df[d] / np.sqrt(var_df[d] + EPS)
        v = (v - mu_df[d]) * Ad + beta_df[d]
    out = v @ np.asarray(Wf, np.float32) + np.asarray(bf, np.float32)
    return out.astype(np.float32)


# revision 44
# speedup vs baseline: 1.0070x; 1.0070x over previous
"""CGCNN forward pass on 8 Trainium2 NeuronCores (Bass/Tile).

Key algebraic identity exploited: the reference uses row = edge_index[0] for
BOTH the gather (h[row]) and the scatter (segment_sum(..., row)), so

    segment_sum(h[row] * ef, row) == h * segment_sum(ef, row)

i.e. the per-edge gather of node features disappears entirely, and the edge
MLP outputs for all 4 conv layers (which depend only on edge_attr) can be
computed in a single pass with stacked weights [64, 4*128].

Sharding: nodes (and the edges that scatter into them) are partitioned into 8
contiguous ranges of 5000 nodes -> one range per core.  Every core's work is
fully independent (no collectives); the only cross-core reduction is the final
mean pool, done on host over 8 [128]-vectors, followed by the (tiny) dense
head in numpy.

Device pipeline per core (edges grouped into 128-edge tiles per 128-node
window, two tiles = one "pair", two pairs = one "block"):
  - EF:    per tile ef_ps[128e, 512] = eaT.T @ Wstack  (two tile_position-
           packed K=64 matmuls per pair when be == 0; K=65 bias-row fallback)
  - evict: ef_pair = relu(ef_ps) cast fp8; Scalar takes one tile, Vector the
           other (swapped each pair) so both engines run every pair and the
           PSUM banks recycle at half the single-engine latency
  - scatter (lagged): blocks of two DoubleRow fp8 matmuls back to back
           (S_psum[128n, 512] += onehot.T @ ef_pair); blocking amortizes the
           256-column DR LDWEIGHTS which cannot hide under EF streams
  - per window: evict S (Scalar), 4x PE-transpose into one [128,512] bf16
    PSUM tile, single strided Vector copy to S^T storage
  - node stage in [h, n] layout: h = W_emb^T x^T (K padded to 128), then 4x
    (hs = h*S_l on Vector bf16 2x; u = Wn_l^T hs (N=512); relu+bias on
    Scalar; affine on GpSimd; h += u on Vector), BN folded into per-partition
    affine A,B.  Mean-pool partial on Vector -> [128,1].
"""

import os
import sys

for _p in ("/opt/trn_rl_repo",):
    if _p not in sys.path and os.path.isdir(_p):
        sys.path.insert(0, _p)

import numpy as np
import ml_dtypes

import concourse.bass as bass  # noqa: F401
import concourse.mybir as mybir
import concourse.tile as tile
from concourse import bacc
from concourse.bass_utils import run_bass_kernel_spmd

N, E = 40000, 640000
NODE_F, EDGE_F, H = 92, 64, 128
L_CONV, L_DENSE = 4, 2
EPS = 1e-3

NCORES = 8
NPC = N // NCORES              # 5000 nodes per core
WIN = 128                      # node window (psum partition dim)
NWIN = (NPC + WIN - 1) // WIN  # 40
NPAD = NWIN * WIN              # 5120
HS = L_CONV * H                # 512 stacked hidden
BF16 = ml_dtypes.bfloat16

_cache = {}


def _host_prep(x, edge_index, edge_attr, W_emb, b_emb, We, be, packed):
    """Bucket+sort edges by destination node, pad to 128-edge tiles per
    128-node window (uniform tile counts across cores, NT even), build
    one-hot tiles and transposed inputs, all bf16."""
    row = np.asarray(edge_index)[0].astype(np.int64)
    ea = np.asarray(edge_attr, np.float32)

    # Degree-balanced node->(core,window) assignment: nodes are permutation-
    # invariant (only the global sum matters), so bin-pack them into the 320
    # buckets (cap 128 nodes) equalizing per-bucket edge counts.  This makes
    # every window need the same tile count (Tw uniform+even -> no window-
    # straddling pairs) and removes ~4% padding.
    import heapq
    deg = np.bincount(row, minlength=N).astype(np.int64)
    NBUCK = NCORES * NWIN
    nodes_by_deg = np.argsort(-deg, kind="stable")
    heap = [(0, b) for b in range(NBUCK)]
    heapq.heapify(heap)
    bucket_of = np.zeros(N, np.int32)
    bucket_cnt = np.zeros(NBUCK, np.int32)
    col_of = np.zeros(N, np.int32)
    spill = []
    for nd in nodes_by_deg:
        s, b = heapq.heappop(heap)
        bucket_of[nd] = b
        col_of[nd] = bucket_cnt[b]
        bucket_cnt[b] += 1
        if bucket_cnt[b] < WIN:
            heapq.heappush(heap, (s + int(deg[nd]), b))
        else:
            spill.append((s + int(deg[nd]), b))
    node_core = (bucket_of // NWIN).astype(np.int64)
    node_win = (bucket_of % NWIN).astype(np.int64)

    core = node_core[row]
    win = node_win[row]
    col = col_of[row].astype(np.int32)
    key = (core * NWIN + win).astype(np.int64)

    order = np.argsort(key, kind="stable")
    counts = np.bincount(key, minlength=NCORES * NWIN).reshape(NCORES, NWIN)
    Tw = np.maximum(np.ceil(counts.max(axis=0) / 128).astype(np.int64), 1)
    if Tw.sum() % 2:
        Tw[-1] += 1  # keep NT even so tiles pair cleanly
    NT = int(Tw.sum())
    EP = NT * 128
    tile_start = np.zeros(NWIN, np.int64)
    tile_start[1:] = np.cumsum(Tw)[:-1]
    slot_start = tile_start * 128

    seg_end = np.cumsum(counts.reshape(-1))
    seg_start = seg_end - counts.reshape(-1)

    ea64 = np.zeros((NCORES, 64, EP), np.float32)
    colslot = np.full((NCORES, EP), -1, np.int32)
    for c in range(NCORES):
        for w in range(NWIN):
            k = c * NWIN + w
            idx = order[seg_start[k]:seg_end[k]]
            n = len(idx)
            if n == 0:
                continue
            s0 = slot_start[w]
            ea64[c, :, s0:s0 + n] = ea[idx].T
            colslot[c, s0:s0 + n] = col[idx]

    if packed:
        # pair layout: rows 0:64 = even tile features, 64:128 = odd tile
        a = ea64.reshape(NCORES, 64, NT, 128)
        eat = np.concatenate(
            [a[:, :, 0::2, :].reshape(NCORES, 64, EP // 2),
             a[:, :, 1::2, :].reshape(NCORES, 64, EP // 2)], axis=1)
    else:
        eat = np.concatenate(
            [ea64, np.ones((NCORES, 1, EP), np.float32)], axis=1)  # bias row

    # one-hot, layout [core, e_in_tile(partition), tile, node_in_window]
    oh = np.zeros((NCORES, 128, NT, 128), np.float32)
    cc, ss = np.nonzero(colslot >= 0)
    t_idx = ss // 128
    e_in = ss - t_idx * 128
    oh[cc, e_in, t_idx, colslot[cc, ss]] = 1.0
    oh = oh.reshape(NCORES, 128, NT * 128)

    # node features transposed + ones row, padded to NPAD; K padded to 128
    xt = np.zeros((NCORES, 128, NPAD), np.float32)
    xf = np.asarray(x, np.float32)
    loc = node_win * WIN + col_of
    xt[node_core, :NODE_F, loc] = xf
    xt[:, 92, :] = 1.0
    nreal = np.bincount(node_core, minlength=NCORES)

    wemb128 = np.zeros((128, 128), np.float32)
    wemb128[:NODE_F] = np.asarray(W_emb, np.float32)
    wemb128[92] = np.asarray(b_emb, np.float32)

    Wef = np.asarray(We, np.float32)
    bef = np.asarray(be, np.float32)
    if packed:
        wstack = np.zeros((128, HS), np.float32)
        for l in range(L_CONV):
            wstack[:64, l * H:(l + 1) * H] = Wef[l]
            wstack[64:, l * H:(l + 1) * H] = Wef[l]
    else:
        wstack = np.zeros((65, HS), np.float32)
        for l in range(L_CONV):
            wstack[:64, l * H:(l + 1) * H] = Wef[l]
            wstack[64, l * H:(l + 1) * H] = bef[l]

    import concourse.mybir as _mb
    F8 = _mb.dt.np(_mb.dt.float8e4)
    return {
        "NT": NT, "Tw": Tw, "EP": EP, "nreal": nreal,
        "eat": np.ascontiguousarray(eat).astype(BF16),
        "oh": oh.astype(F8),
        "xt": xt.astype(BF16),
        "wemb": wemb128.astype(BF16),
        "wstack": wstack.astype(BF16),
    }


def _build_program(NT, Tw, packed):
    from concourse.masks import make_identity

    EP = NT * 128
    f32 = mybir.dt.float32
    bf = mybir.dt.bfloat16
    f8 = mybir.dt.float8e4
    Relu = mybir.ActivationFunctionType.Relu
    MULT = mybir.AluOpType.mult
    ADD = mybir.AluOpType.add

    nc = bacc.Bacc(None, target_bir_lowering=False)
    d_eat = nc.dram_tensor(
        "eat", [128 if packed else 65, EP // 2 if packed else EP], bf,
        kind="ExternalInput")
    d_oh = nc.dram_tensor("oh", [128, EP], f8, kind="ExternalInput")
    d_xt = nc.dram_tensor("xt", [128, NPAD], bf, kind="ExternalInput")
    d_wemb = nc.dram_tensor("wemb", [128, 128], bf, kind="ExternalInput")
    d_wstack = nc.dram_tensor(
        "wstack", [128 if packed else 65, HS], bf, kind="ExternalInput")
    d_wn = nc.dram_tensor("wn", [128, HS], bf, kind="ExternalInput")
    d_bnb = nc.dram_tensor("bnb", [128, L_CONV], f32, kind="ExternalInput")
    d_ab = nc.dram_tensor("ab", [128, L_CONV], f32, kind="ExternalInput")
    d_abn = nc.dram_tensor("abn", [128, L_CONV], f32, kind="ExternalInput")
    d_bb = nc.dram_tensor("bb", [128, L_CONV], bf, kind="ExternalInput")
    d_out = nc.dram_tensor("hsum", [128, 1], f32, kind="ExternalOutput")

    NPAIR = NT // 2
    # node-chunk schedule: 512-wide chunks, the last 512 split into 128s to
    # shorten the serial dependency chain in the kernel tail
    CHUNKS = []
    _ci = 0
    _n = 0
    while _n < NPAD:
        w_ = 128 if _n >= NPAD - 512 else 512
        CHUNKS.append((_ci, _n, _n + w_))
        _ci += 1
        _n += w_
    N_CHUNKS = len(CHUNKS)
    GP = 8                      # pairs per DMA group (16 tiles)
    LAG = 10                    # scatter lags LAG pairs
    FLUSH = 4                   # scatter block size (pairs per flush)

    # window id per tile
    win_of = np.repeat(np.arange(NWIN), Tw)
    first_of = np.zeros(NT, bool)
    last_of = np.zeros(NT, bool)
    pos = 0
    for w in range(NWIN):
        first_of[pos] = True
        last_of[pos + int(Tw[w]) - 1] = True
        pos += int(Tw[w])

    with tile.TileContext(nc) as tc:
        with (
            tc.tile_pool(name="const", bufs=1) as constp,
            tc.tile_pool(name="ea", bufs=4) as eap,
            tc.tile_pool(name="ohp", bufs=4) as ohp,
            tc.tile_pool(name="ef", bufs=20) as efp,
            tc.tile_pool(name="swin", bufs=3) as swinp,
            tc.tile_pool(name="efps", bufs=4, space="PSUM") as efpsp,
            tc.tile_pool(name="sps", bufs=2, space="PSUM") as spsp,
            tc.tile_pool(name="big", bufs=1) as bigp,
        ):
            # first edge-data slice goes FIRST so the PE can start ~5us
            # earlier; everything else queues behind it
            gs0 = min(GP, NPAIR)
            ea_t0 = eap.tile(
                [128, GP * 128] if packed else [65, GP * 256], bf,
                tag="ea", name="ea_t0")
            h0 = min(1, gs0)
            if packed:
                nc.sync.dma_start(ea_t0[:, :h0 * 128], d_eat[:, :h0 * 128])
            else:
                nc.sync.dma_start(ea_t0[:, :h0 * 256], d_eat[:, :h0 * 256])
            wstack_sb = constp.tile([128 if packed else 65, HS], bf)
            nc.sync.dma_start(wstack_sb[:], d_wstack[:])
            oh_t0 = ohp.tile([128, GP * 256], f8, tag="oh", name="oh_t0")
            nc.sync.dma_start(oh_t0[:, :h0 * 256], d_oh[:, :h0 * 256])
            if packed and gs0 > h0:
                nc.sync.dma_start(
                    ea_t0[:, h0 * 128:gs0 * 128],
                    d_eat[:, h0 * 128:gs0 * 128])
            elif gs0 > h0:
                nc.sync.dma_start(
                    ea_t0[:, h0 * 256:gs0 * 256],
                    d_eat[:, h0 * 256:gs0 * 256])
            if gs0 > h0:
                nc.sync.dma_start(
                    oh_t0[:, h0 * 256:gs0 * 256],
                    d_oh[:, h0 * 256:gs0 * 256])
            wemb_sb = constp.tile([128, 128], bf)
            nc.sync.dma_start(wemb_sb[:], d_wemb[:])
            wn_sb = constp.tile([128, HS], bf)
            nc.sync.dma_start(wn_sb[:], d_wn[:])
            bnb_sb = constp.tile([128, L_CONV], f32)
            nc.sync.dma_start(bnb_sb[:], d_bnb[:])
            ab_sb = constp.tile([128, L_CONV], f32)
            nc.sync.dma_start(ab_sb[:], d_ab[:])
            abn_sb = constp.tile([128, L_CONV], f32)
            nc.sync.dma_start(abn_sb[:], d_abn[:])
            bb_sb = constp.tile([128, L_CONV], bf)
            nc.sync.dma_start(bb_sb[:], d_bb[:])
            ident_sb = constp.tile([128, 128], bf)
            make_identity(nc, ident_sb[:])

            xt_sb = bigp.tile([128, NPAD], bf)
            NK = NPAD // 512
            st_sb = bigp.tile([128, L_CONV * NPAD], bf)   # S^T per layer
            h_sb = bigp.tile([128, NPAD], bf)
            hs_sb = bigp.tile([128, NPAD], bf)
            t_sb = bigp.tile([128, NPAD], bf)
            u_sb = bigp.tile([128, NPAD], bf)
            hsum_sb = bigp.tile([128, 1], f32)
            hpart_sb = bigp.tile([128, N_CHUNKS], f32)

            node_chains = []
            tr_queue = []
            pop_rr = [0]

            def make_emb(n0, n1):
                def s_emb():
                    cw = n1 - n0
                    h_ps = spsp.tile(
                        [128, 512], f32, tag="nodeps", bufs=1, name="h_ps")
                    nc.tensor.matmul(
                        h_ps[:, :cw], wemb_sb[:], xt_sb[:, n0:n1],
                        start=True, stop=True)
                    nc.scalar.copy(h_sb[:, n0:n1], h_ps[:, :cw])
                return s_emb

            def pop_steps(n):
                for _ in range(n):
                    # transposes drain first: chunk chains depend on their
                    # windows' S^T being written (program order guarantees it)
                    if tr_queue:
                        tr_queue.pop(0)()
                        continue
                    if not node_chains:
                        return
                    idx = pop_rr[0] % min(2, len(node_chains))
                    chain = node_chains[idx]
                    chain.pop(0)()
                    if not chain:
                        node_chains.pop(idx)
                    pop_rr[0] += 1

            def queue_node_chunk(k, n0, n1):
                """Embed + all 4 conv layers for node cols [n0, n1), as a
                list of small steps drip-fed so dependent chains never
                block the in-order engine FIFOs.  hs on Vector (bf16 2x),
                relu(+folded BN scale) on Scalar, h update fused on
                Vector, matmuls on TensorE."""
                sl = slice(n0, n1)
                cw = n1 - n0
                chain = [make_emb(n0, n1)]

                def s_mul(l):
                    def f():
                        nc.vector.tensor_tensor(
                            hs_sb[:, sl], h_sb[:, sl],
                            st_sb[:, l * NPAD + n0:l * NPAD + n1],
                            op=MULT)
                    return f

                def s_mm(l):
                    def f():
                        u_ps = spsp.tile(
                            [128, 512], f32, tag="nodeps", bufs=1,
                            name="u_ps")
                        nc.tensor.matmul(
                            u_ps[:, :cw], wn_sb[:, l * 128:(l + 1) * 128],
                            hs_sb[:, sl], start=True, stop=True)
                        # A>0 so A*relu(u+bn) == relu(A*u + A*bn): BN scale
                        # folds into the relu eviction for free
                        nc.scalar.activation(
                            t_sb[:, sl], u_ps[:, :cw], Relu,
                            bias=abn_sb[:, l:l + 1],
                            scale=ab_sb[:, l:l + 1])
                    return f

                def s_stt(l):
                    def f():
                        # h += t + B in one fused op; the final layer also
                        # emits the chunk's mean-pool partial via accum_out
                        nc.vector.scalar_tensor_tensor(
                            h_sb[:, sl], t_sb[:, sl], bb_sb[:, l:l + 1],
                            h_sb[:, sl], op0=ADD, op1=ADD,
                            accum_out=(hpart_sb[:, k:k + 1]
                                       if l == L_CONV - 1 else None))
                    return f

                for l in range(L_CONV):
                    chain.append(s_mul(l))
                    chain.append(s_mm(l))
                    chain.append(s_stt(l))

                node_chains.append(chain)

            # ---------------- edge stage (software-pipelined) ------------
            state = {"s_ps": None}
            ef_tiles = [None] * NPAIR
            oh_groups = [None] * NPAIR

            def scatter_pair_dr(q, ef_pair, oh_g, goff):
                """Both tiles of pair q in one fp8 DoubleRow matmul
                (contracts 256 edges at 2 MAC/cell/cycle)."""
                t0 = 2 * q
                if first_of[t0]:
                    state["s_ps"] = spsp.tile(
                        [128, HS], f32, tag="sps", name="s_ps")
                s_ps = state["s_ps"]
                j = t0 - goff
                oh3 = oh_g[:, j * 128:(j + 2) * 128].rearrange(
                    "p (two n) -> p two n", two=2)
                ef3 = ef_pair[:].rearrange("p (two n) -> p two n", two=2)
                nc.tensor.matmul(
                    s_ps[:], oh3, ef3,
                    start=bool(first_of[t0]), stop=bool(last_of[t0 + 1]),
                    perf_mode=mybir.MatmulPerfMode.DoubleRow)
                finish_window(t0 + 1)

            def emit_scatter(t, ef_pair, oh_g, goff):
                if first_of[t]:
                    state["s_ps"] = spsp.tile(
                        [128, HS], f32, tag="sps", name="s_ps")
                s_ps = state["s_ps"]
                j = t - goff
                nc.tensor.matmul(
                    s_ps[:], oh_g[:, j * 128:(j + 1) * 128],
                    ef_pair[:, (t % 2) * HS:(t % 2 + 1) * HS],
                    start=bool(first_of[t]), stop=bool(last_of[t]))
                finish_window(t)

            def finish_window(t):
                w = int(win_of[t])
                s_ps = state["s_ps"]
                if last_of[t]:
                    s_sb = swinp.tile([128, HS], bf, tag="swin")
                    nc.scalar.copy(s_sb[:], s_ps[:])

                    def s_tr(s_sb, w, l, tr_box):
                        def f():
                            if l == 0:
                                tr_box[0] = spsp.tile(
                                    [128, 512], bf, tag="trps", bufs=1,
                                    name="tr_ps")
                            tr_ps = tr_box[0]
                            nc.tensor.transpose(
                                tr_ps[:, l * 128:(l + 1) * 128],
                                s_sb[:, l * 128:(l + 1) * 128],
                                ident_sb[:])
                            if l == L_CONV - 1:
                                # one strided copy moves all 4 transposed
                                # layers into S^T storage
                                dst = st_sb[:].rearrange(
                                    "p (l m) -> p l m", l=L_CONV)[
                                    :, :, w * WIN:(w + 1) * WIN]
                                nc.vector.tensor_copy(dst, tr_ps[:].rearrange(
                                    "p (l n) -> p l n", l=L_CONV))
                        return f
                    tr_box = [None]
                    tr_queue.extend(
                        [s_tr(s_sb, w, l, tr_box) for l in range(L_CONV)])
                    done = (w + 1) * WIN
                    while CHUNKS and CHUNKS[0][2] <= done:
                        queue_node_chunk(*CHUNKS.pop(0))

            parity = 0
            g0p = 0
            ns_box = [0]
            pend_scatter = []
            group_tiles = {}

            def load_group(p0):
                if p0 >= NPAIR or p0 in group_tiles:
                    return
                gs = min(GP, NPAIR - p0)
                ea_t = eap.tile(
                    [128, GP * 128] if packed else [65, GP * 256], bf,
                    tag="ea", name="ea_t")
                if packed:
                    nc.sync.dma_start(
                        ea_t[:, :gs * 128],
                        d_eat[:, p0 * 128:(p0 + gs) * 128])
                else:
                    nc.sync.dma_start(
                        ea_t[:, :gs * 256],
                        d_eat[:, p0 * 256:(p0 + gs) * 256])
                oh_t = ohp.tile([128, GP * 256], f8, tag="oh", name="oh_t")
                nc.sync.dma_start(
                    oh_t[:, :gs * 256], d_oh[:, p0 * 256:(p0 + gs) * 256])
                group_tiles[p0] = (ea_t, oh_t)

            group_tiles[0] = (ea_t0, oh_t0)
            load_group(GP)
            for p in range(NPAIR):
                if p < NK:
                    nc.sync.dma_start(
                        xt_sb[:, p * 512:(p + 1) * 512],
                        d_xt[:, p * 512:(p + 1) * 512])
                if p % GP == 0:
                    g0p = p
                    ea_g, oh_g = group_tiles[p]
                    load_group(p + 2 * GP)   # keep two groups in flight

                jp = p - g0p
                ps_a = efpsp.tile([128, HS], f32, tag="efps", name="ef_psa")
                ps_b = efpsp.tile([128, HS], f32, tag="efps", name="ef_psb")
                if packed:
                    pe = ea_g[:, jp * 128:(jp + 1) * 128]
                    nc.tensor.matmul(
                        ps_a[:], pe[0:64, :], wstack_sb[0:64, :],
                        start=True, stop=True, tile_position=(0, 0))
                    nc.tensor.matmul(
                        ps_b[:], pe[64:128, :],
                        wstack_sb[64:128, :],
                        start=True, stop=True, tile_position=(64, 0))
                else:
                    nc.tensor.matmul(
                        ps_a[:], ea_g[:, jp * 256:jp * 256 + 128],
                        wstack_sb[:], start=True, stop=True)
                    nc.tensor.matmul(
                        ps_b[:],
                        ea_g[:, jp * 256 + 128:(jp + 1) * 256],
                        wstack_sb[:], start=True, stop=True)

                ef_pair = efp.tile([128, 2 * HS], f8, tag="ef")
                # both engines evict every pair (one tile each, swapping
                # roles) so PSUM banks free at half the single-engine latency
                if parity:
                    nc.scalar.activation(ef_pair[:, 0:HS], ps_a[:], Relu)
                    nc.vector.tensor_scalar_max(
                        ef_pair[:, HS:2 * HS], ps_b[:], 0.0)
                else:
                    nc.vector.tensor_scalar_max(ef_pair[:, 0:HS], ps_a[:], 0.0)
                    nc.scalar.activation(ef_pair[:, HS:2 * HS], ps_b[:], Relu)
                parity ^= 1
                ef_tiles[p] = ef_pair
                oh_groups[p] = (oh_g, 2 * g0p)

                lag_eff = 3 if (p < 12 or p >= NPAIR - 6) else LAG
                while ns_box[0] <= p - lag_eff:
                    pend_scatter.append(ns_box[0])
                    ns_box[0] += 1
                flushed = False
                if len(pend_scatter) >= (
                        FLUSH if p < NPAIR - 6 else 2) or (
                        p == NPAIR - 1 and pend_scatter):
                    for q in pend_scatter:
                        og, goff = oh_groups[q]
                        if win_of[2 * q] == win_of[2 * q + 1]:
                            scatter_pair_dr(q, ef_tiles[q], og, goff)
                        else:
                            emit_scatter(2 * q, ef_tiles[q], og, goff)
                            emit_scatter(2 * q + 1, ef_tiles[q], og, goff)
                        ef_tiles[q] = None
                    pend_scatter = []
                    flushed = True
                # burst node steps right after scatter blocks so their PE
                # matmuls cluster with the DR run instead of breaking up EF
                if flushed or NPAIR - p < 60:
                    pop_steps(FLUSH + 1 if flushed else 2)
            for q in range(ns_box[0], NPAIR):
                og, goff = oh_groups[q]
                if win_of[2 * q] == win_of[2 * q + 1]:
                    scatter_pair_dr(q, ef_tiles[q], og, goff)
                else:
                    emit_scatter(2 * q, ef_tiles[q], og, goff)
                    emit_scatter(2 * q + 1, ef_tiles[q], og, goff)
            pop_steps(10 ** 9)

            nc.vector.tensor_reduce(
                hsum_sb[:], hpart_sb[:], axis=mybir.AxisListType.X,
                op=ADD)
            nc.sync.dma_start(d_out[:], hsum_sb[:])

    nc.finalize()
    return nc


LAST_EXEC_NS = None
LAST_RESULT = None


def kernel(x, edge_index, edge_attr, W_emb, b_emb, We, be, Wn, bn,
           g_c, beta_c, mu_c, var_c, Wd, bd, g_d, beta_d, mu_d, var_d, Wf, bf):
    global LAST_EXEC_NS

    packed = bool(np.all(np.asarray(be) == 0.0))
    prep = _host_prep(x, edge_index, edge_attr, W_emb, b_emb, We, be, packed)
    NT, Tw = prep["NT"], prep["Tw"]

    Wnf = np.asarray(Wn, np.float32)
    wn_stack = np.zeros((128, HS), np.float32)
    for l in range(L_CONV):
        wn_stack[:, l * H:(l + 1) * H] = Wnf[l]
    A = (np.asarray(g_c, np.float32)
         / np.sqrt(np.asarray(var_c, np.float32) + EPS))        # [L, H]
    B = np.asarray(beta_c, np.float32) - np.asarray(mu_c, np.float32) * A

    key = (NT, tuple(int(v) for v in Tw), packed)
    if key not in _cache:
        _cache[key] = _build_program(NT, Tw, packed)
    nc = _cache[key]

    common = {
        "wemb": prep["wemb"],
        "wstack": prep["wstack"],
        "wn": wn_stack.astype(BF16),
        "bnb": np.ascontiguousarray(np.asarray(bn, np.float32).T).reshape(128, L_CONV),
        "ab": np.ascontiguousarray(A.T).reshape(128, L_CONV),
        "abn": np.ascontiguousarray((A * np.asarray(bn, np.float32)).T).reshape(128, L_CONV),
        "bb": np.ascontiguousarray(B.T).reshape(128, L_CONV).astype(BF16),
    }
    in_maps = []
    for c in range(NCORES):
        m = dict(common)
        m["eat"] = prep["eat"][c]
        m["oh"] = prep["oh"][c]
        m["xt"] = prep["xt"][c]
        in_maps.append(m)

    trace = bool(os.environ.get("KERNEL_TRACE"))
    if trace:
        try:
            from trn_agent_boot.trn_boot import _ntff_profile_via_ctypes
            from antenv.axon_hooks import set_axon_ntff_profile_hook
            set_axon_ntff_profile_hook(
                _ntff_profile_via_ctypes("/opt/axon/libaxon_pjrt.so"))
        except Exception:
            trace = False

    res = run_bass_kernel_spmd(
        nc, in_maps, core_ids=list(range(NCORES)), trace=trace)
    LAST_EXEC_NS = res.exec_time_ns
    global LAST_RESULT
    LAST_RESULT = res

    total = np.zeros(128, np.float64)
    for c in range(NCORES):
        total += res.results[c]["hsum"].reshape(128).astype(np.float64)
    # padded (fake) node columns contribute a closed-form constant: S=0 there
    # so h_fake = b_emb + sum_l (A_l*relu(bn_l) + B_l)
    h_fake = np.asarray(b_emb, np.float64).copy()
    for l in range(L_CONV):
        h_fake = h_fake + A[l].astype(np.float64) * np.maximum(
            np.asarray(bn, np.float64)[l], 0.0) + B[l].astype(np.float64)
    n_fake = NCORES * NPAD - N
    total -= n_fake * h_fake
    v = (total / N).astype(np.float32)

    # dense head on host (0.000001% of total FLOPs)
    g_df = np.asarray(g_d, np.float32)
    var_df = np.asarray(var_d, np.float32)
    beta_df = np.asarray(beta_d, np.float32)
    mu_df = np.asarray(mu_d, np.float32)
    Wdf = np.asarray(Wd, np.float32)
    bdf = np.asarray(bd, np.float32)
    for d in range(L_DENSE):
        v = np.maximum(v @ Wdf[d] + bdf[d], 0.0)
        Ad = g_df[d] / np.sqrt(var_df[d] + EPS)
        v = (v - mu_df[d]) * Ad + beta_df[d]
    out = v @ np.asarray(Wf, np.float32) + np.asarray(bf, np.float32)
    return out.astype(np.float32)


# revision 45
# speedup vs baseline: 1.0083x; 1.0013x over previous
"""CGCNN forward pass on 8 Trainium2 NeuronCores (Bass/Tile).

Key algebraic identity exploited: the reference uses row = edge_index[0] for
BOTH the gather (h[row]) and the scatter (segment_sum(..., row)), so

    segment_sum(h[row] * ef, row) == h * segment_sum(ef, row)

i.e. the per-edge gather of node features disappears entirely, and the edge
MLP outputs for all 4 conv layers (which depend only on edge_attr) can be
computed in a single pass with stacked weights [64, 4*128].

Sharding: nodes (and the edges that scatter into them) are partitioned into 8
contiguous ranges of 5000 nodes -> one range per core.  Every core's work is
fully independent (no collectives); the only cross-core reduction is the final
mean pool, done on host over 8 [128]-vectors, followed by the (tiny) dense
head in numpy.

Device pipeline per core (edges grouped into 128-edge tiles per 128-node
window, two tiles = one "pair", two pairs = one "block"):
  - EF:    per tile ef_ps[128e, 512] = eaT.T @ Wstack  (two tile_position-
           packed K=64 matmuls per pair when be == 0; K=65 bias-row fallback)
  - evict: ef_pair = relu(ef_ps) cast fp8; Scalar takes one tile, Vector the
           other (swapped each pair) so both engines run every pair and the
           PSUM banks recycle at half the single-engine latency
  - scatter (lagged): blocks of two DoubleRow fp8 matmuls back to back
           (S_psum[128n, 512] += onehot.T @ ef_pair); blocking amortizes the
           256-column DR LDWEIGHTS which cannot hide under EF streams
  - per window: evict S (Scalar), 4x PE-transpose into one [128,512] bf16
    PSUM tile, single strided Vector copy to S^T storage
  - node stage in [h, n] layout: h = W_emb^T x^T (K padded to 128), then 4x
    (hs = h*S_l on Vector bf16 2x; u = Wn_l^T hs (N=512); relu+bias on
    Scalar; affine on GpSimd; h += u on Vector), BN folded into per-partition
    affine A,B.  Mean-pool partial on Vector -> [128,1].
"""

import os
import sys

for _p in ("/opt/trn_rl_repo",):
    if _p not in sys.path and os.path.isdir(_p):
        sys.path.insert(0, _p)

import numpy as np
import ml_dtypes

import concourse.bass as bass  # noqa: F401
import concourse.mybir as mybir
import concourse.tile as tile
from concourse import bacc
from concourse.bass_utils import run_bass_kernel_spmd

N, E = 40000, 640000
NODE_F, EDGE_F, H = 92, 64, 128
L_CONV, L_DENSE = 4, 2
EPS = 1e-3

NCORES = 8
NPC = N // NCORES              # 5000 nodes per core
WIN = 128                      # node window (psum partition dim)
NWIN = (NPC + WIN - 1) // WIN  # 40
NPAD = NWIN * WIN              # 5120
HS = L_CONV * H                # 512 stacked hidden
BF16 = ml_dtypes.bfloat16

_cache = {}


def _host_prep(x, edge_index, edge_attr, W_emb, b_emb, We, be, packed):
    """Bucket+sort edges by destination node, pad to 128-edge tiles per
    128-node window (uniform tile counts across cores, NT even), build
    one-hot tiles and transposed inputs, all bf16."""
    row = np.asarray(edge_index)[0].astype(np.int64)
    ea = np.asarray(edge_attr, np.float32)

    # Degree-balanced node->(core,window) assignment: nodes are permutation-
    # invariant (only the global sum matters), so bin-pack them into the 320
    # buckets (cap 128 nodes) equalizing per-bucket edge counts.  This makes
    # every window need the same tile count (Tw uniform+even -> no window-
    # straddling pairs) and removes ~4% padding.
    import heapq
    deg = np.bincount(row, minlength=N).astype(np.int64)
    NBUCK = NCORES * NWIN
    nodes_by_deg = np.argsort(-deg, kind="stable")
    heap = [(0, b) for b in range(NBUCK)]
    heapq.heapify(heap)
    bucket_of = np.zeros(N, np.int32)
    bucket_cnt = np.zeros(NBUCK, np.int32)
    col_of = np.zeros(N, np.int32)
    spill = []
    for nd in nodes_by_deg:
        s, b = heapq.heappop(heap)
        bucket_of[nd] = b
        col_of[nd] = bucket_cnt[b]
        bucket_cnt[b] += 1
        if bucket_cnt[b] < WIN:
            heapq.heappush(heap, (s + int(deg[nd]), b))
        else:
            spill.append((s + int(deg[nd]), b))
    node_core = (bucket_of // NWIN).astype(np.int64)
    node_win = (bucket_of % NWIN).astype(np.int64)

    core = node_core[row]
    win = node_win[row]
    col = col_of[row].astype(np.int32)
    key = (core * NWIN + win).astype(np.int64)

    order = np.argsort(key, kind="stable")
    counts = np.bincount(key, minlength=NCORES * NWIN).reshape(NCORES, NWIN)
    Tw = np.maximum(np.ceil(counts.max(axis=0) / 128).astype(np.int64), 1)
    if Tw.sum() % 2:
        Tw[-1] += 1  # keep NT even so tiles pair cleanly
    NT = int(Tw.sum())
    EP = NT * 128
    tile_start = np.zeros(NWIN, np.int64)
    tile_start[1:] = np.cumsum(Tw)[:-1]
    slot_start = tile_start * 128

    seg_end = np.cumsum(counts.reshape(-1))
    seg_start = seg_end - counts.reshape(-1)

    ea64 = np.zeros((NCORES, 64, EP), np.float32)
    colslot = np.full((NCORES, EP), -1, np.int32)
    for c in range(NCORES):
        for w in range(NWIN):
            k = c * NWIN + w
            idx = order[seg_start[k]:seg_end[k]]
            n = len(idx)
            if n == 0:
                continue
            s0 = slot_start[w]
            ea64[c, :, s0:s0 + n] = ea[idx].T
            colslot[c, s0:s0 + n] = col[idx]

    if packed:
        # pair layout: rows 0:64 = even tile features, 64:128 = odd tile
        a = ea64.reshape(NCORES, 64, NT, 128)
        eat = np.concatenate(
            [a[:, :, 0::2, :].reshape(NCORES, 64, EP // 2),
             a[:, :, 1::2, :].reshape(NCORES, 64, EP // 2)], axis=1)
    else:
        eat = np.concatenate(
            [ea64, np.ones((NCORES, 1, EP), np.float32)], axis=1)  # bias row

    # one-hot, layout [core, e_in_tile(partition), tile, node_in_window]
    oh = np.zeros((NCORES, 128, NT, 128), np.float32)
    cc, ss = np.nonzero(colslot >= 0)
    t_idx = ss // 128
    e_in = ss - t_idx * 128
    oh[cc, e_in, t_idx, colslot[cc, ss]] = 1.0
    oh = oh.reshape(NCORES, 128, NT * 128)

    # node features transposed + ones row, padded to NPAD; K padded to 128
    xt = np.zeros((NCORES, 128, NPAD), np.float32)
    xf = np.asarray(x, np.float32)
    loc = node_win * WIN + col_of
    xt[node_core, :NODE_F, loc] = xf
    xt[:, 92, :] = 1.0
    nreal = np.bincount(node_core, minlength=NCORES)

    wemb128 = np.zeros((128, 128), np.float32)
    wemb128[:NODE_F] = np.asarray(W_emb, np.float32)
    wemb128[92] = np.asarray(b_emb, np.float32)

    Wef = np.asarray(We, np.float32)
    bef = np.asarray(be, np.float32)
    if packed:
        wstack = np.zeros((128, HS), np.float32)
        for l in range(L_CONV):
            wstack[:64, l * H:(l + 1) * H] = Wef[l]
            wstack[64:, l * H:(l + 1) * H] = Wef[l]
    else:
        wstack = np.zeros((65, HS), np.float32)
        for l in range(L_CONV):
            wstack[:64, l * H:(l + 1) * H] = Wef[l]
            wstack[64, l * H:(l + 1) * H] = bef[l]

    import concourse.mybir as _mb
    F8 = _mb.dt.np(_mb.dt.float8e4)
    return {
        "NT": NT, "Tw": Tw, "EP": EP, "nreal": nreal,
        "eat": np.ascontiguousarray(eat).astype(BF16),
        "oh": oh.astype(F8),
        "xt": xt.astype(BF16),
        "wemb": wemb128.astype(BF16),
        "wstack": wstack.astype(BF16),
    }


def _build_program(NT, Tw, packed):
    from concourse.masks import make_identity

    EP = NT * 128
    f32 = mybir.dt.float32
    bf = mybir.dt.bfloat16
    f8 = mybir.dt.float8e4
    Relu = mybir.ActivationFunctionType.Relu
    MULT = mybir.AluOpType.mult
    ADD = mybir.AluOpType.add

    nc = bacc.Bacc(None, target_bir_lowering=False)
    d_eat = nc.dram_tensor(
        "eat", [128 if packed else 65, EP // 2 if packed else EP], bf,
        kind="ExternalInput")
    d_oh = nc.dram_tensor("oh", [128, EP], f8, kind="ExternalInput")
    d_xt = nc.dram_tensor("xt", [128, NPAD], bf, kind="ExternalInput")
    d_wemb = nc.dram_tensor("wemb", [128, 128], bf, kind="ExternalInput")
    d_wstack = nc.dram_tensor(
        "wstack", [128 if packed else 65, HS], bf, kind="ExternalInput")
    d_wn = nc.dram_tensor("wn", [128, HS], bf, kind="ExternalInput")
    d_bnb = nc.dram_tensor("bnb", [128, L_CONV], f32, kind="ExternalInput")
    d_ab = nc.dram_tensor("ab", [128, L_CONV], f32, kind="ExternalInput")
    d_abn = nc.dram_tensor("abn", [128, L_CONV], f32, kind="ExternalInput")
    d_bb = nc.dram_tensor("bb", [128, L_CONV], bf, kind="ExternalInput")
    d_out = nc.dram_tensor("hsum", [128, 1], f32, kind="ExternalOutput")

    NPAIR = NT // 2
    # node-chunk schedule: 512-wide chunks, the last 512 split into 128s to
    # shorten the serial dependency chain in the kernel tail
    CHUNKS = []
    _ci = 0
    _n = 0
    while _n < NPAD:
        w_ = 128 if _n >= NPAD - 512 else 512
        CHUNKS.append((_ci, _n, _n + w_))
        _ci += 1
        _n += w_
    N_CHUNKS = len(CHUNKS)
    GP = 8                      # pairs per DMA group (16 tiles)
    LAG = 10                    # scatter lags LAG pairs
    FLUSH = 4                   # scatter block size (pairs per flush)

    # window id per tile
    win_of = np.repeat(np.arange(NWIN), Tw)
    first_of = np.zeros(NT, bool)
    last_of = np.zeros(NT, bool)
    pos = 0
    for w in range(NWIN):
        first_of[pos] = True
        last_of[pos + int(Tw[w]) - 1] = True
        pos += int(Tw[w])

    with tile.TileContext(nc) as tc:
        with (
            tc.tile_pool(name="const", bufs=1) as constp,
            tc.tile_pool(name="ea", bufs=4) as eap,
            tc.tile_pool(name="ohp", bufs=4) as ohp,
            tc.tile_pool(name="ef", bufs=20) as efp,
            tc.tile_pool(name="swin", bufs=3) as swinp,
            tc.tile_pool(name="efps", bufs=4, space="PSUM") as efpsp,
            tc.tile_pool(name="sps", bufs=2, space="PSUM") as spsp,
            tc.tile_pool(name="big", bufs=1) as bigp,
        ):
            # first edge-data slice goes FIRST so the PE can start ~5us
            # earlier; everything else queues behind it
            gs0 = min(GP, NPAIR)
            ea_t0 = eap.tile(
                [128, GP * 128] if packed else [65, GP * 256], bf,
                tag="ea", name="ea_t0")
            h0 = min(1, gs0)
            if packed:
                nc.sync.dma_start(ea_t0[:, :h0 * 128], d_eat[:, :h0 * 128])
            else:
                nc.sync.dma_start(ea_t0[:, :h0 * 256], d_eat[:, :h0 * 256])
            wstack_sb = constp.tile([128 if packed else 65, HS], bf)
            nc.sync.dma_start(wstack_sb[:], d_wstack[:])
            oh_t0 = ohp.tile([128, GP * 256], f8, tag="oh", name="oh_t0")
            nc.sync.dma_start(oh_t0[:, :h0 * 256], d_oh[:, :h0 * 256])
            if packed and gs0 > h0:
                nc.sync.dma_start(
                    ea_t0[:, h0 * 128:gs0 * 128],
                    d_eat[:, h0 * 128:gs0 * 128])
            elif gs0 > h0:
                nc.sync.dma_start(
                    ea_t0[:, h0 * 256:gs0 * 256],
                    d_eat[:, h0 * 256:gs0 * 256])
            if gs0 > h0:
                nc.sync.dma_start(
                    oh_t0[:, h0 * 256:gs0 * 256],
                    d_oh[:, h0 * 256:gs0 * 256])
            wemb_sb = constp.tile([128, 128], bf)
            nc.sync.dma_start(wemb_sb[:], d_wemb[:])
            wn_sb = constp.tile([128, HS], bf)
            nc.sync.dma_start(wn_sb[:], d_wn[:])
            bnb_sb = constp.tile([128, L_CONV], f32)
            nc.sync.dma_start(bnb_sb[:], d_bnb[:])
            ab_sb = constp.tile([128, L_CONV], f32)
            nc.sync.dma_start(ab_sb[:], d_ab[:])
            abn_sb = constp.tile([128, L_CONV], f32)
            nc.sync.dma_start(abn_sb[:], d_abn[:])
            bb_sb = constp.tile([128, L_CONV], bf)
            nc.sync.dma_start(bb_sb[:], d_bb[:])
            ident_sb = constp.tile([128, 128], bf)
            make_identity(nc, ident_sb[:])

            xt_sb = bigp.tile([128, NPAD], bf)
            NK = NPAD // 512
            st_sb = bigp.tile([128, L_CONV * NPAD], bf)   # S^T per layer
            h_sb = bigp.tile([128, NPAD], bf)
            hs_sb = bigp.tile([128, NPAD], bf)
            t_sb = bigp.tile([128, NPAD], bf)
            u_sb = bigp.tile([128, NPAD], bf)
            hsum_sb = bigp.tile([128, 1], f32)
            hpart_sb = bigp.tile([128, N_CHUNKS], f32)

            node_chains = []
            tr_queue = []
            pop_rr = [0]

            def make_emb(n0, n1):
                def s_emb():
                    cw = n1 - n0
                    h_ps = spsp.tile(
                        [128, 512], f32, tag="nodeps", bufs=1, name="h_ps")
                    nc.tensor.matmul(
                        h_ps[:, :cw], wemb_sb[:], xt_sb[:, n0:n1],
                        start=True, stop=True)
                    nc.scalar.copy(h_sb[:, n0:n1], h_ps[:, :cw])
                return s_emb

            def pop_steps(n):
                for _ in range(n):
                    # transposes drain first: chunk chains depend on their
                    # windows' S^T being written (program order guarantees it)
                    if tr_queue:
                        tr_queue.pop(0)()
                        continue
                    if not node_chains:
                        return
                    idx = pop_rr[0] % min(2, len(node_chains))
                    chain = node_chains[idx]
                    chain.pop(0)()
                    if not chain:
                        node_chains.pop(idx)
                    pop_rr[0] += 1

            def queue_node_chunk(k, n0, n1):
                """Embed + all 4 conv layers for node cols [n0, n1), as a
                list of small steps drip-fed so dependent chains never
                block the in-order engine FIFOs.  hs on Vector (bf16 2x),
                relu(+folded BN scale) on Scalar, h update fused on
                Vector, matmuls on TensorE."""
                sl = slice(n0, n1)
                cw = n1 - n0
                chain = [make_emb(n0, n1)]

                def s_mul(l):
                    def f():
                        nc.vector.tensor_tensor(
                            hs_sb[:, sl], h_sb[:, sl],
                            st_sb[:, l * NPAD + n0:l * NPAD + n1],
                            op=MULT)
                    return f

                def s_mm(l):
                    def f():
                        u_ps = spsp.tile(
                            [128, 512], f32, tag="nodeps", bufs=1,
                            name="u_ps")
                        nc.tensor.matmul(
                            u_ps[:, :cw], wn_sb[:, l * 128:(l + 1) * 128],
                            hs_sb[:, sl], start=True, stop=True)
                        # A>0 so A*relu(u+bn) == relu(A*u + A*bn): BN scale
                        # folds into the relu eviction for free
                        nc.scalar.activation(
                            t_sb[:, sl], u_ps[:, :cw], Relu,
                            bias=abn_sb[:, l:l + 1],
                            scale=ab_sb[:, l:l + 1])
                    return f

                def s_stt(l):
                    def f():
                        # h += t + B in one fused op; the final layer also
                        # emits the chunk's mean-pool partial via accum_out
                        nc.vector.scalar_tensor_tensor(
                            h_sb[:, sl], t_sb[:, sl], bb_sb[:, l:l + 1],
                            h_sb[:, sl], op0=ADD, op1=ADD,
                            accum_out=(hpart_sb[:, k:k + 1]
                                       if l == L_CONV - 1 else None))
                    return f

                for l in range(L_CONV):
                    chain.append(s_mul(l))
                    chain.append(s_mm(l))
                    chain.append(s_stt(l))

                node_chains.append(chain)

            # ---------------- edge stage (software-pipelined) ------------
            state = {"s_ps": None}
            ef_tiles = [None] * NPAIR
            oh_groups = [None] * NPAIR

            def scatter_pair_dr(q, ef_pair, oh_g, goff):
                """Both tiles of pair q in one fp8 DoubleRow matmul
                (contracts 256 edges at 2 MAC/cell/cycle)."""
                t0 = 2 * q
                if first_of[t0]:
                    state["s_ps"] = spsp.tile(
                        [128, HS], f32, tag="sps", name="s_ps")
                s_ps = state["s_ps"]
                j = t0 - goff
                oh3 = oh_g[:, j * 128:(j + 2) * 128].rearrange(
                    "p (two n) -> p two n", two=2)
                ef3 = ef_pair[:].rearrange("p (two n) -> p two n", two=2)
                nc.tensor.matmul(
                    s_ps[:], oh3, ef3,
                    start=bool(first_of[t0]), stop=bool(last_of[t0 + 1]),
                    perf_mode=mybir.MatmulPerfMode.DoubleRow)
                finish_window(t0 + 1)

            def emit_scatter(t, ef_pair, oh_g, goff):
                if first_of[t]:
                    state["s_ps"] = spsp.tile(
                        [128, HS], f32, tag="sps", name="s_ps")
                s_ps = state["s_ps"]
                j = t - goff
                nc.tensor.matmul(
                    s_ps[:], oh_g[:, j * 128:(j + 1) * 128],
                    ef_pair[:, (t % 2) * HS:(t % 2 + 1) * HS],
                    start=bool(first_of[t]), stop=bool(last_of[t]))
                finish_window(t)

            def finish_window(t):
                w = int(win_of[t])
                s_ps = state["s_ps"]
                if last_of[t]:
                    s_sb = swinp.tile([128, HS], bf, tag="swin")
                    nc.scalar.copy(s_sb[:], s_ps[:])

                    def s_tr(s_sb, w, l, tr_box):
                        def f():
                            if l == 0:
                                tr_box[0] = spsp.tile(
                                    [128, 512], bf, tag="trps", bufs=1,
                                    name="tr_ps")
                            tr_ps = tr_box[0]
                            nc.tensor.transpose(
                                tr_ps[:, l * 128:(l + 1) * 128],
                                s_sb[:, l * 128:(l + 1) * 128],
                                ident_sb[:])
                            if l == L_CONV - 1:
                                # one strided copy moves all 4 transposed
                                # layers into S^T storage
                                dst = st_sb[:].rearrange(
                                    "p (l m) -> p l m", l=L_CONV)[
                                    :, :, w * WIN:(w + 1) * WIN]
                                nc.vector.tensor_copy(dst, tr_ps[:].rearrange(
                                    "p (l n) -> p l n", l=L_CONV))
                        return f
                    tr_box = [None]
                    tr_queue.extend(
                        [s_tr(s_sb, w, l, tr_box) for l in range(L_CONV)])
                    done = (w + 1) * WIN
                    while CHUNKS and CHUNKS[0][2] <= done:
                        queue_node_chunk(*CHUNKS.pop(0))

            parity = 0
            g0p = 0
            ns_box = [0]
            pend_scatter = []
            group_tiles = {}

            def load_group(p0):
                if p0 >= NPAIR or p0 in group_tiles:
                    return
                gs = min(GP, NPAIR - p0)
                ea_t = eap.tile(
                    [128, GP * 128] if packed else [65, GP * 256], bf,
                    tag="ea", name="ea_t")
                if packed:
                    nc.sync.dma_start(
                        ea_t[:, :gs * 128],
                        d_eat[:, p0 * 128:(p0 + gs) * 128])
                else:
                    nc.sync.dma_start(
                        ea_t[:, :gs * 256],
                        d_eat[:, p0 * 256:(p0 + gs) * 256])
                oh_t = ohp.tile([128, GP * 256], f8, tag="oh", name="oh_t")
                nc.sync.dma_start(
                    oh_t[:, :gs * 256], d_oh[:, p0 * 256:(p0 + gs) * 256])
                group_tiles[p0] = (ea_t, oh_t)

            group_tiles[0] = (ea_t0, oh_t0)
            load_group(GP)
            for p in range(NPAIR):
                # xt is first needed by the chunk-0 embed around pair ~26;
                # loading it after the ramp keeps early DMA bandwidth for the
                # edge-data groups
                if 8 <= p < NK + 8:
                    kx = p - 8
                    nc.sync.dma_start(
                        xt_sb[:, kx * 512:(kx + 1) * 512],
                        d_xt[:, kx * 512:(kx + 1) * 512])
                if p % GP == 0:
                    g0p = p
                    ea_g, oh_g = group_tiles[p]
                    load_group(p + 2 * GP)   # keep two groups in flight

                jp = p - g0p
                ps_a = efpsp.tile([128, HS], f32, tag="efps", name="ef_psa")
                ps_b = efpsp.tile([128, HS], f32, tag="efps", name="ef_psb")
                if packed:
                    pe = ea_g[:, jp * 128:(jp + 1) * 128]
                    nc.tensor.matmul(
                        ps_a[:], pe[0:64, :], wstack_sb[0:64, :],
                        start=True, stop=True, tile_position=(0, 0))
                    nc.tensor.matmul(
                        ps_b[:], pe[64:128, :],
                        wstack_sb[64:128, :],
                        start=True, stop=True, tile_position=(64, 0))
                else:
                    nc.tensor.matmul(
                        ps_a[:], ea_g[:, jp * 256:jp * 256 + 128],
                        wstack_sb[:], start=True, stop=True)
                    nc.tensor.matmul(
                        ps_b[:],
                        ea_g[:, jp * 256 + 128:(jp + 1) * 256],
                        wstack_sb[:], start=True, stop=True)

                ef_pair = efp.tile([128, 2 * HS], f8, tag="ef")
                # both engines evict every pair (one tile each, swapping
                # roles) so PSUM banks free at half the single-engine latency
                if parity:
                    nc.scalar.activation(ef_pair[:, 0:HS], ps_a[:], Relu)
                    nc.vector.tensor_scalar_max(
                        ef_pair[:, HS:2 * HS], ps_b[:], 0.0)
                else:
                    nc.vector.tensor_scalar_max(ef_pair[:, 0:HS], ps_a[:], 0.0)
                    nc.scalar.activation(ef_pair[:, HS:2 * HS], ps_b[:], Relu)
                parity ^= 1
                ef_tiles[p] = ef_pair
                oh_groups[p] = (oh_g, 2 * g0p)

                lag_eff = 3 if (p < 12 or p >= NPAIR - 6) else LAG
                while ns_box[0] <= p - lag_eff:
                    pend_scatter.append(ns_box[0])
                    ns_box[0] += 1
                flushed = False
                if len(pend_scatter) >= (
                        FLUSH if p < NPAIR - 6 else 2) or (
                        p == NPAIR - 1 and pend_scatter):
                    for q in pend_scatter:
                        og, goff = oh_groups[q]
                        if win_of[2 * q] == win_of[2 * q + 1]:
                            scatter_pair_dr(q, ef_tiles[q], og, goff)
                        else:
                            emit_scatter(2 * q, ef_tiles[q], og, goff)
                            emit_scatter(2 * q + 1, ef_tiles[q], og, goff)
                        ef_tiles[q] = None
                    pend_scatter = []
                    flushed = True
                # burst node steps right after scatter blocks so their PE
                # matmuls cluster with the DR run instead of breaking up EF
                if flushed or NPAIR - p < 60:
                    pop_steps(FLUSH + 1 if flushed else 2)
            for q in range(ns_box[0], NPAIR):
                og, goff = oh_groups[q]
                if win_of[2 * q] == win_of[2 * q + 1]:
                    scatter_pair_dr(q, ef_tiles[q], og, goff)
                else:
                    emit_scatter(2 * q, ef_tiles[q], og, goff)
                    emit_scatter(2 * q + 1, ef_tiles[q], og, goff)
            pop_steps(10 ** 9)

            nc.vector.tensor_reduce(
                hsum_sb[:], hpart_sb[:], axis=mybir.AxisListType.X,
                op=ADD)
            nc.sync.dma_start(d_out[:], hsum_sb[:])

    nc.finalize()
    return nc


LAST_EXEC_NS = None
LAST_RESULT = None


def kernel(x, edge_index, edge_attr, W_emb, b_emb, We, be, Wn, bn,
           g_c, beta_c, mu_c, var_c, Wd, bd, g_d, beta_d, mu_d, var_d, Wf, bf):
    global LAST_EXEC_NS

    packed = bool(np.all(np.asarray(be) == 0.0))
    prep = _host_prep(x, edge_index, edge_attr, W_emb, b_emb, We, be, packed)
    NT, Tw = prep["NT"], prep["Tw"]

    Wnf = np.asarray(Wn, np.float32)
    wn_stack = np.zeros((128, HS), np.float32)
    for l in range(L_CONV):
        wn_stack[:, l * H:(l + 1) * H] = Wnf[l]
    A = (np.asarray(g_c, np.float32)
         / np.sqrt(np.asarray(var_c, np.float32) + EPS))        # [L, H]
    B = np.asarray(beta_c, np.float32) - np.asarray(mu_c, np.float32) * A

    key = (NT, tuple(int(v) for v in Tw), packed)
    if key not in _cache:
        _cache[key] = _build_program(NT, Tw, packed)
    nc = _cache[key]

    common = {
        "wemb": prep["wemb"],
        "wstack": prep["wstack"],
        "wn": wn_stack.astype(BF16),
        "bnb": np.ascontiguousarray(np.asarray(bn, np.float32).T).reshape(128, L_CONV),
        "ab": np.ascontiguousarray(A.T).reshape(128, L_CONV),
        "abn": np.ascontiguousarray((A * np.asarray(bn, np.float32)).T).reshape(128, L_CONV),
        "bb": np.ascontiguousarray(B.T).reshape(128, L_CONV).astype(BF16),
    }
    in_maps = []
    for c in range(NCORES):
        m = dict(common)
        m["eat"] = prep["eat"][c]
        m["oh"] = prep["oh"][c]
        m["xt"] = prep["xt"][c]
        in_maps.append(m)

    trace = bool(os.environ.get("KERNEL_TRACE"))
    if trace:
        try:
            from trn_agent_boot.trn_boot import _ntff_profile_via_ctypes
            from antenv.axon_hooks import set_axon_ntff_profile_hook
            set_axon_ntff_profile_hook(
                _ntff_profile_via_ctypes("/opt/axon/libaxon_pjrt.so"))
        except Exception:
            trace = False

    res = run_bass_kernel_spmd(
        nc, in_maps, core_ids=list(range(NCORES)), trace=trace)
    LAST_EXEC_NS = res.exec_time_ns
    global LAST_RESULT
    LAST_RESULT = res

    total = np.zeros(128, np.float64)
    for c in range(NCORES):
        total += res.results[c]["hsum"].reshape(128).astype(np.float64)
    # padded (fake) node columns contribute a closed-form constant: S=0 there
    # so h_fake = b_emb + sum_l (A_l*relu(bn_l) + B_l)
    h_fake = np.asarray(b_emb, np.float64).copy()
    for l in range(L_CONV):
        h_fake = h_fake + A[l].astype(np.float64) * np.maximum(
            np.asarray(bn, np.float64)[l], 0.0) + B[l].astype(np.float64)
    n_fake = NCORES * NPAD - N
    total -= n_fake * h_fake
    v = (total / N).astype(np.float32)

    # dense head on host (0.000001% of total FLOPs)
    g_df = np.asarray(g_d, np.float32)
    var_df = np.asarray(var_d, np.float32)
    beta_df = np.asarray(beta_d, np.float32)
    mu_df = np.asarray(mu_d, np.float32)
    Wdf = np.asarray(Wd, np.float32)
    bdf = np.asarray(bd, np.float32)
    for d in range(L_DENSE):
        v = np.maximum(v @ Wdf[d] + bdf[d], 0.0)
        Ad = g_df[d] / np.sqrt(var_df[d] + EPS)
        v = (v - mu_df[d]) * Ad + beta_df[d]
    out = v @ np.asarray(Wf, np.float32) + np.asarray(bf, np.float32)
    return out.astype(np.float32)
